# revision 2
# baseline (speedup 1.0000x reference)
"""AxialSpaceTimeTransformer on 8 TRN2 NeuronCores (Bass + XLA hybrid).

Sharding (8-way, single chip):
  * t-domain: core c holds frames t in [4c, 4c+4) for both batches.
    Space-attention (over s) and FF are core-local here.
  * s-domain: core c holds spatial positions s in [32c, 32c+32).
    Causal time-attention (over t) is core-local here.
Resharding between domains is one 8-rank all_to_all (on-device).

The six space layers (0-2, 4-6) — ~75% of FLOPs — run as a hand-written
Bass kernel (float32r matmuls, fused norm/softcap/softmax-renorm) invoked
twice as a bass_exec custom call. The two time layers, value-residual
projection, final norm and the all_to_alls run as XLA programs on the
same cores; everything chains device-resident.
"""

import os
import sys
import types

import numpy as np

if "/opt/trn_rl_repo" not in sys.path:
    sys.path.insert(0, "/opt/trn_rl_repo")

# -- antenv.axon_hooks shim (agent image lacks it; bass_utils wants it) --
import antenv  # noqa: E402

if not hasattr(antenv, "axon_hooks"):
    _hooks = types.ModuleType("antenv.axon_hooks")
    _hooks._hook = None
    _hooks.set_axon_ntff_profile_hook = lambda h: setattr(_hooks, "_hook", h)
    _hooks.get_axon_ntff_profile_hook = lambda: _hooks._hook
    sys.modules["antenv.axon_hooks"] = _hooks
    antenv.axon_hooks = _hooks
    try:
        from trn_agent_boot.trn_boot import _ntff_profile_via_ctypes

        _hooks.set_axon_ntff_profile_hook(
            _ntff_profile_via_ctypes("/opt/axon/libaxon_pjrt.so")
        )
    except Exception:
        pass

import jax  # noqa: E402
import jax.numpy as jnp  # noqa: E402
from jax.sharding import Mesh, NamedSharding, PartitionSpec as P  # noqa: E402
from jax.experimental.shard_map import shard_map  # noqa: E402

DIM = 768
DEPTH = 8
HEADS = 12
DH = 64
DFF = 2048
SOFTCLAMP = 50.0
B, T, S = 2, 32, 256
EPS = 1e-6
NC = 8
TL = T // NC  # 4 frames/core (t-domain)
SL = S // NC  # 32 positions/core (s-domain)
NTOK = B * TL * S  # 2048 tokens per core in either domain

USE_BASS = os.environ.get("KERNEL_NO_BASS", "0") != "1"


def _round_f32r(x):
    """fp32 -> fp32r (13 explicit mantissa bits, RNE) rounding on host."""
    u = np.ascontiguousarray(x, dtype=np.float32).view(np.uint32)
    lsb = (u >> 10) & 1
    r = (u + 0x1FF + lsb) & np.uint32(0xFFFFFC00)
    return r.view(np.float32).copy()


def _rmsnorm(x):
    return x * jax.lax.rsqrt(jnp.mean(x * x, axis=-1, keepdims=True) + EPS)


def _l2norm(x):
    n = jnp.sqrt(jnp.sum(x * x, axis=-1, keepdims=True))
    return x / jnp.maximum(n, 1e-12)


def _make_rotary(n):
    inv = 1.0 / (10000.0 ** (np.arange(0, DH, 2, dtype=np.float32) / DH))
    f = np.arange(n, dtype=np.float32)[:, None] * inv[None, :]
    return np.concatenate([f, f], axis=-1)  # (n, DH)


def _t2s(x):
    """per-core t-domain (B*TL, S, *d) -> s-domain (B*SL, T, *d)."""
    d = x.shape[2:]
    x5 = jnp.moveaxis(x.reshape(B, TL, NC, SL, *d), 2, 0)  # (sblk,b,tl,sl,d)
    y = jax.lax.all_to_all(x5, "core", split_axis=0, concat_axis=0, tiled=True)
    # y: (tblk, b, tl, sl, d) -> (b, sl, tblk, tl, d)
    y = y.transpose(1, 3, 0, 2, *range(4, 4 + len(d)))
    return y.reshape(B * SL, T, *d)


def _s2t(x):
    """per-core s-domain (B*SL, T, *d) -> t-domain (B*TL, S, *d)."""
    d = x.shape[2:]
    x5 = jnp.moveaxis(x.reshape(B, SL, NC, TL, *d), 2, 0)  # (tblk,b,sl,tl,d)
    y = jax.lax.all_to_all(x5, "core", split_axis=0, concat_axis=0, tiled=True)
    # y: (sblk, b, sl, tl, d) -> (b, tl, sblk, sl, d)
    y = y.transpose(1, 3, 0, 2, *range(4, 4 + len(d)))
    return y.reshape(B * TL, S, *d)


def _time_attn_ff(x, rv, w, rot, kgam):
    """One causal time layer + FF on per-core s-domain data (XLA)."""
    n = x.shape[1]
    tn = _rmsnorm(x)  # norm weights folded into w host-side
    q = (tn @ w["Wq"]).reshape(-1, n, HEADS, DH).transpose(0, 2, 1, 3)
    k = (tn @ w["Wk"]).reshape(-1, n, HEADS, DH).transpose(0, 2, 1, 3)
    v = (tn @ w["Wv"]).reshape(-1, n, HEADS, DH).transpose(0, 2, 1, 3)
    rva = rv.reshape(-1, n, HEADS, DH).transpose(0, 2, 1, 3)
    mix = jax.nn.sigmoid(tn @ w["Wmix"] + w["bmix"]).transpose(0, 2, 1)[..., None]
    v = v + mix * (rva - v)
    k = _l2norm(k) * ((kgam + 1.0) * (DH**0.5))[:, None, :]
    cosr = jnp.cos(rot)
    sinr = jnp.sin(rot)

    def rotate(xx):
        x1, x2 = jnp.split(xx, 2, axis=-1)
        return xx * cosr + jnp.concatenate([-x2, x1], axis=-1) * sinr

    q = rotate(q)
    k = rotate(k)
    sim = jnp.einsum("bhid,bhjd->bhij", q, k) * (DH**-0.5)
    sim = jnp.tanh(sim / SOFTCLAMP) * SOFTCLAMP
    cm = jnp.triu(jnp.ones((n, n), dtype=bool), 1)
    sim = jnp.where(cm, -jnp.finfo(sim.dtype).max, sim)
    attn = jax.nn.softmax(sim, axis=-1)
    o = jnp.einsum("bhij,bhjd->bhid", attn, v)
    gates = jax.nn.sigmoid(tn @ w["Wg"]).transpose(0, 2, 1)[..., None]
    o = (o * gates).transpose(0, 2, 1, 3).reshape(-1, n, HEADS * DH)
    x = x + o @ w["Wo"]
    tn2 = _rmsnorm(x)
    h = tn2 @ w["Win"] + w["b_in"]
    a, g = jnp.split(h, 2, axis=-1)
    x = x + (a * jax.nn.gelu(g, approximate=False)) @ w["Wout"] + w["b_out"]
    return x


def _space_stack_jax(x, rv, ws, kgs):
    """Fallback XLA implementation of 3 space layers (t-domain)."""
    n = x.shape[1]
    rva = rv.reshape(-1, n, HEADS, DH).transpose(0, 2, 1, 3)
    for w, kgam in zip(ws, kgs):
        tn = _rmsnorm(x)
        q = (tn @ w["Wq"]).reshape(-1, n, HEADS, DH).transpose(0, 2, 1, 3)
        k = (tn @ w["Wk"]).reshape(-1, n, HEADS, DH).transpose(0, 2, 1, 3)
        v = (tn @ w["Wv"]).reshape(-1, n, HEADS, DH).transpose(0, 2, 1, 3)
        mix = jax.nn.sigmoid(tn @ w["Wmix"] + w["bmix"]).transpose(0, 2, 1)[..., None]
        v = v + mix * (rva - v)
        k = _l2norm(k) * ((kgam + 1.0) * (DH**0.5))[:, None, :]
        sim = jnp.einsum("bhid,bhjd->bhij", q, k) * (DH**-0.5)
        sim = jnp.tanh(sim / SOFTCLAMP) * SOFTCLAMP
        attn = jax.nn.softmax(sim, axis=-1)
        o = jnp.einsum("bhij,bhjd->bhid", attn, v)
        gates = jax.nn.sigmoid(tn @ w["Wg"]).transpose(0, 2, 1)[..., None]
        o = (o * gates).transpose(0, 2, 1, 3).reshape(-1, n, HEADS * DH)
        x = x + o @ w["Wo"]
        tn2 = _rmsnorm(x)
        h = tn2 @ w["Win"] + w["b_in"]
        a, g = jnp.split(h, 2, axis=-1)
        x = x + (a * jax.nn.gelu(g, approximate=False)) @ w["Wout"] + w["b_out"]
    return x


# ---------------------------------------------------------------------------
# cached compiled pipeline
# ---------------------------------------------------------------------------
_PIPE = None


def _layer_w(inputs, i, fold_norm=True):
    """Per-layer weight dict with norm weights folded in (host)."""
    f32 = np.float32
    anw = np.asarray(inputs["attn_norm_w"][i], f32)[:, None]
    fnw = np.asarray(inputs["ff_norm_w"][i], f32)[:, None]
    return {
        "Wq": jnp.asarray(np.asarray(inputs["Wq"][i], f32) * anw),
        "Wk": jnp.asarray(np.asarray(inputs["Wk"][i], f32) * anw),
        "Wv": jnp.asarray(np.asarray(inputs["Wv"][i], f32) * anw),
        "Wmix": jnp.asarray(np.asarray(inputs["Wmix"][i], f32) * anw),
        "Wg": jnp.asarray(np.asarray(inputs["Wg"][i], f32) * anw),
        "bmix": jnp.asarray(np.asarray(inputs["bmix"][i], f32)),
        "Wo": jnp.asarray(np.asarray(inputs["Wo"][i], f32)),
        "Win": jnp.asarray(np.asarray(inputs["Win"][i], f32) * fnw),
        "b_in": jnp.asarray(np.asarray(inputs["b_in"][i], f32)),
        "Wout": jnp.asarray(np.asarray(inputs["Wout"][i], f32)),
        "b_out": jnp.asarray(np.asarray(inputs["b_out"][i], f32)),
    }


def _bass_pack(inputs, layers):
    """Stacked, f32r-rounded weights for one bass_space3 call (np)."""
    f32 = np.float32
    idx = list(layers)
    anw = np.asarray(inputs["attn_norm_w"], f32)[idx][:, :, None]
    fnw = np.asarray(inputs["ff_norm_w"], f32)[idx][:, :, None]
    g = {}
    g["Wq3"] = _round_f32r(np.asarray(inputs["Wq"], f32)[idx] * anw)
    g["Wk3"] = _round_f32r(np.asarray(inputs["Wk"], f32)[idx] * anw)
    g["Wv3"] = _round_f32r(np.asarray(inputs["Wv"], f32)[idx] * anw)
    g["Wo3"] = _round_f32r(np.asarray(inputs["Wo"], f32)[idx])
    g["Wmg3"] = _round_f32r(
        np.concatenate(
            [
                np.asarray(inputs["Wmix"], f32)[idx] * anw,
                np.asarray(inputs["Wg"], f32)[idx] * anw,
            ],
            axis=2,
        )
    )  # (3, 768, 24)
    # k scale applied after l2norm; folds sqrt(DH), 1/sqrt(DH) and 1/softclamp
    g["kg3"] = (
        ((np.asarray(inputs["k_gamma"], f32)[idx] + 1.0) / SOFTCLAMP)
        .reshape(3, HEADS * DH)
        .astype(f32)
    )
    g["Win3"] = _round_f32r(np.asarray(inputs["Win"], f32)[idx] * fnw)
    g["Wout3"] = _round_f32r(np.asarray(inputs["Wout"], f32)[idx])
    return g


def _build_pipeline(inputs):
    devs = jax.devices()[:NC]
    mesh = Mesh(np.asarray(devs), ("core",))
    shard = NamedSharding(mesh, P("core"))
    repl = NamedSharding(mesh, P())

    vrW = jnp.asarray(
        np.asarray(inputs["vr_norm_w"], np.float32)[:, None]
        * np.asarray(inputs["vr_W"], np.float32)
    )
    w3 = _layer_w(inputs, 3)
    w7 = _layer_w(inputs, 7)
    kg3 = jnp.asarray(np.asarray(inputs["k_gamma"][3], np.float32))
    kg7 = jnp.asarray(np.asarray(inputs["k_gamma"][7], np.float32))
    rot = jnp.asarray(_make_rotary(T))

    # ---- stage 1: rv + reshard rv to s-domain --------------------------
    def f_pre(tok):
        rv = _rmsnorm(tok) @ vrW  # (B*TL, S, 768)
        rv_s = _t2s(rv)  # (B*SL, T, 768)
        return tok.reshape(NTOK, DIM), rv.reshape(NTOK, DIM), rv_s

    pre = jax.jit(
        shard_map(f_pre, mesh=mesh, in_specs=(P("core"),),
                  out_specs=(P("core"),) * 3, check_rep=False)
    )

    # ---- stage 2: time layer (mid: reshard in and out; last: + final) --
    def f_time_mid(x_t, rv_s, w, kgam):
        x = _t2s(x_t.reshape(B * TL, S, DIM))
        x = _time_attn_ff(x, rv_s.reshape(B * SL * T, DIM).reshape(B * SL, T, DIM),
                          w, rot, kgam)
        return _s2t(x).reshape(NTOK, DIM)

    def f_time_last(x_t, rv_s, w, kgam):
        x = _t2s(x_t.reshape(B * TL, S, DIM))
        x = _time_attn_ff(x, rv_s, w, rot, kgam)
        return _rmsnorm(x)  # (B*SL, T, DIM); final_norm_w applied on host

    wspec = jax.tree_util.tree_map(lambda _: P(), w3)
    tmid = jax.jit(
        shard_map(f_time_mid, mesh=mesh,
                  in_specs=(P("core"), P("core"), wspec, P()),
                  out_specs=P("core"), check_rep=False)
    )
    tlast = jax.jit(
        shard_map(f_time_last, mesh=mesh,
                  in_specs=(P("core"), P("core"), wspec, P()),
                  out_specs=P("core"), check_rep=False)
    )

    # ---- space stacks ---------------------------------------------------
    if USE_BASS:
        from bass_space3 import build_space3

        nc, in_names, out_names, out_avals = build_space3()
        from concourse import bass2jax
        from concourse.bass2jax import _bass_exec_p

        bind_names = tuple(in_names + out_names)

        def bass_body(*args):
            ops = list(args)
            if nc.partition_id_tensor is not None:
                ops.append(bass2jax.partition_id_tensor())
            outs = _bass_exec_p.bind(
                *ops,
                out_avals=tuple(out_avals),
                in_names=bind_names,
                out_names=tuple(out_names),
                lowering_input_output_aliases=(),
                sim_require_finite=True,
                sim_require_nnan=True,
                nc=nc,
            )
            return tuple(outs)

        # operand sharding: per-core tensors sharded, weights replicated
        percore = {"x_in", "rv_in", "x_out"}
        in_specs = tuple(
            P("core") if n in percore else P() for n in bind_names
        )
        out_specs = (P("core"),) * len(out_names)
        nout = len(out_names)
        bass_jit = jax.jit(
            shard_map(bass_body, mesh=mesh, in_specs=in_specs,
                      out_specs=out_specs, check_rep=False),
            donate_argnums=tuple(
                range(len(bind_names) - nout, len(bind_names))
            ),
        )

        packs = [
            {k: jnp.asarray(v) for k, v in _bass_pack(inputs, [0, 1, 2]).items()},
            {k: jnp.asarray(v) for k, v in _bass_pack(inputs, [4, 5, 6]).items()},
        ]

        def space_stack(x_flat, rv_flat, which):
            pk = packs[which]
            ops = []
            for nme in in_names:
                if nme == "x_in":
                    ops.append(x_flat)
                elif nme == "rv_in":
                    ops.append(rv_flat)
                else:
                    ops.append(pk[nme])
            zeros = jnp.zeros((NC * NTOK, DIM), jnp.float32)
            zeros = jax.device_put(zeros, shard)
            (out,) = bass_jit(*ops, zeros)
            return out
    else:
        ws_a = [_layer_w(inputs, i) for i in (0, 1, 2)]
        ws_b = [_layer_w(inputs, i) for i in (4, 5, 6)]
        kgs_a = [jnp.asarray(np.asarray(inputs["k_gamma"][i], np.float32))
                 for i in (0, 1, 2)]
        kgs_b = [jnp.asarray(np.asarray(inputs["k_gamma"][i], np.float32))
                 for i in (4, 5, 6)]

        def f_space(x_flat, rv_flat, ws, kgs):
            x = _space_stack_jax(
                x_flat.reshape(B * TL, S, DIM), rv_flat.reshape(B * TL, S, DIM),
                ws, kgs,
            )
            return x.reshape(NTOK, DIM)

        wsspec = jax.tree_util.tree_map(lambda _: P(), ws_a)
        kgspec = jax.tree_util.tree_map(lambda _: P(), kgs_a)
        sjit = jax.jit(
            shard_map(f_space, mesh=mesh,
                      in_specs=(P("core"), P("core"), wsspec, kgspec),
                      out_specs=P("core"), check_rep=False)
        )

        def space_stack(x_flat, rv_flat, which):
            ws, kgs = (ws_a, kgs_a) if which == 0 else (ws_b, kgs_b)
            return sjit(x_flat, rv_flat, ws, kgs)

    fnw = jnp.asarray(np.asarray(inputs["final_norm_w"], np.float32))

    def run(tok_bt):
        tok = jax.device_put(tok_bt, shard)
        x_flat, rv_flat, rv_s = pre(tok)
        x_flat = space_stack(x_flat, rv_flat, 0)
        x_flat = tmid(x_flat, rv_s, w3, kg3)
        x_flat = space_stack(x_flat, rv_flat, 1)
        out = tlast(x_flat, rv_s, w7, kg7)
        return out

    return run


def kernel(**inputs):
    global _PIPE
    tokens = np.asarray(inputs["tokens"], dtype=np.float32)
    # global (NC*B*TL, S, DIM): rows (c, b, tl) -> t = 4c + tl
    tok_bt = np.ascontiguousarray(
        tokens.transpose(1, 0, 2, 3)
        .reshape(NC, TL, B, S, DIM)
        .transpose(0, 2, 1, 3, 4)
    ).reshape(NC * B * TL, S, DIM)

    if _PIPE is None:
        _PIPE = _build_pipeline(inputs)
    out = np.asarray(jax.block_until_ready(_PIPE(jnp.asarray(tok_bt))))

    # out: (NC*B*SL, T, DIM), rows (c, b, sl) with s = 32c + sl
    out = out.reshape(NC, B, SL, T, DIM).transpose(1, 3, 0, 2, 4)
    out = out.reshape(B, T, S, DIM)
    out = out * np.asarray(inputs["final_norm_w"], np.float32)
    return np.ascontiguousarray(out.astype(np.float32))
